# revision 11
# baseline (speedup 1.0000x reference)
"""Trainium2 Bass kernel for nn_MetaRL_LightGAT_BiACT (GAT + LayerNorm + MLP).

Strategy (8 NeuronCores, row-sharded, transposed layout [j_part, i_free]):
  - Each core owns 1024 of the 8192 output rows (node dim N=i); the full
    j dim (8192) is reduced on-chip via PSUM accumulation.
  - Host precomputes the tiny GAT projection Wh = x @ W_gat.T and scores
    s = Wh @ a.T (0.15% of FLOPs), and marshals adj into a single bf16
    tensor  adjm[j, i] = adj[i, j] ? s_i : -60   (pre-transposed and
    pre-tiled so each superchunk is one contiguous [128, sc*1024] DMA slab).
  - Identity used on device, per element (w = adjm):
        adj * exp(prelu(s_i + s_j))
      = exp(max(w, 0.2*w - 0.8*s_j) + s_j)            (w = s_i on edges)
      = exp(max(w, 0.2*w - 0.8*s_j)) * e^{s_j}
    with e^{s_j} folded into the matmul weights WhU[j,:] = e^{s_j}*Wh[j,:]
    (and the softmax-denominator ones column becomes e^{s_j}).
    Non-edges (w = -60) give exp(<= -11) ~ 0, i.e. the mask.
  - Device main loop per superchunk (512 j's):
      DVE/GpSimd tensor_scalar (bf16): t1 = 0.2*w - 0.8*s_j (per 128-chunk;
           split across both engines to keep DVE under the ACT roofline)
      DVE  tensor_tensor (bf16 2x): q = max(w, t1)        (whole slab)
      ACT  activation    Exp       : q = exp(q)           (whole slab)
      PE   matmul bf16: acc[65, i] += WhU_chunk^T @ q_chunk  (PSUM accum,
           col 64 of WhU is e^{s_j} -> softmax denominator D for free)
  - Epilogue (per 512-col half): LayerNorm is invariant to the positive
    per-column scale 1/D except through eps:
        (h'-mu)/sqrt(var+eps) = (num - mu_num) / sqrt(var_num + eps*D^2)
    so the attention normalization is never applied explicitly. The
    [65, 512] accumulator is PE-transposed to natural [i_part, d] layout
    where all per-i stats (sums, sqrt, reciprocal) vectorize across 128
    lanes, gamma/beta are folded into W1/b1 on host, and the
    48->256->128->32 MLP runs in bf16 after transposing back. The final
    [32, 512] result is DMA'd out transposed; the host un-transposes.
"""

import sys

if "/opt/trn_rl_repo" not in sys.path:
    sys.path.insert(0, "/opt/trn_rl_repo")

import numpy as np
import ml_dtypes

N = 8192
D_IN = 128
D_H = 48
D_AUG = 65  # WhU cols 0-47, zeros 48-63, e^{s_j} col at 64
D_OUT = 32
N_CORES = 8
ROWS = N // N_CORES          # 1024 rows per core
P = 128                      # partitions
SC_CHUNKS = 4                # j-chunks per superchunk
MASK_VAL = -60.0
EPS = 1e-5
GP_CHUNKS = 2                # TS chunks per superchunk routed to GpSimd


def build_nc(num_cores=N_CORES, rows=ROWS, n=N, reps=1, gp_chunks=GP_CHUNKS,
             prefetch=2):
    import concourse.bass as bass
    import concourse.mybir as mybir
    import concourse.tile as tile
    from concourse import bacc
    from concourse.masks import make_identity
    from contextlib import ExitStack

    f32 = mybir.dt.float32
    bf16 = mybir.dt.bfloat16
    AF = mybir.ActivationFunctionType
    OP = mybir.AluOpType
    AX = mybir.AxisListType

    n_chunk = n // P
    n_sc = max(1, n_chunk // SC_CHUNKS)
    sc_chunks = n_chunk // n_sc
    n_half = rows // 512

    nc = bacc.Bacc("TRN2", target_bir_lowering=False, debug=False,
                   num_devices=num_cores)

    adjm_d = nc.dram_tensor("adjm", [n_sc * P, sc_chunks * rows], bf16,
                            kind="ExternalInput").ap()
    whu_d = nc.dram_tensor("whu", [P, n_chunk * D_AUG], bf16,
                           kind="ExternalInput").ap()
    sJm_d = nc.dram_tensor("sJm", [P, n_chunk], f32, kind="ExternalInput").ap()
    w1g_d = nc.dram_tensor("w1g", [D_H, 256], bf16, kind="ExternalInput").ap()
    b1_d = nc.dram_tensor("b1", [256, 1], f32, kind="ExternalInput").ap()
    w2t_d = nc.dram_tensor("w2t", [256, 128], bf16, kind="ExternalInput").ap()
    b2_d = nc.dram_tensor("b2", [128, 1], f32, kind="ExternalInput").ap()
    w3t_d = nc.dram_tensor("w3t", [128, D_OUT], bf16, kind="ExternalInput").ap()
    b3_d = nc.dram_tensor("b3", [D_OUT, 1], f32, kind="ExternalInput").ap()
    out_d = nc.dram_tensor("out", [n_half, D_OUT, 512], f32,
                           kind="ExternalOutput").ap()

    with ExitStack() as ctx:
        tc = ctx.enter_context(tile.TileContext(nc))
        singles = ctx.enter_context(tc.tile_pool(name="singles", bufs=1))
        adjp = ctx.enter_context(tc.tile_pool(name="adjp", bufs=3))
        t1p = ctx.enter_context(tc.tile_pool(name="t1p", bufs=2))
        qp = ctx.enter_context(tc.tile_pool(name="qp", bufs=2))
        hp = ctx.enter_context(tc.tile_pool(name="hp", bufs=2))

        # sJm first (needed by the very first TS), then prefetch the first
        # adjm slabs on the Sync queue before any other singles traffic.
        sJm_sb = singles.tile([P, n_chunk], f32)
        nc.scalar.dma_start(sJm_sb, sJm_d)
        pre_adjm = {}
        for sc in range(min(prefetch, n_sc)):
            adjm = adjp.tile([P, sc_chunks, rows], bf16, name=f"adjm{sc}")
            nc.sync.dma_start(adjm.rearrange("p a b -> p (a b)"),
                              adjm_d[sc * P:(sc + 1) * P, :])
            pre_adjm[sc] = adjm

        # remaining resident small tensors (scalar HWDGE queue)
        whu_sb = singles.tile([P, n_chunk, D_AUG], bf16)
        nc.scalar.dma_start(whu_sb, whu_d.rearrange("p (c d) -> p c d",
                                                    d=D_AUG))
        w1g_sb = singles.tile([D_H, 256], bf16)
        nc.scalar.dma_start(w1g_sb, w1g_d)
        w2t_sb = singles.tile([P, 2, 128], bf16)
        nc.scalar.dma_start(w2t_sb, w2t_d.rearrange("(m p) k -> p m k", p=P))
        w3t_sb = singles.tile([P, D_OUT], bf16)
        nc.scalar.dma_start(w3t_sb, w3t_d)
        b1_sb = singles.tile([P, 2], f32)
        nc.scalar.dma_start(b1_sb, b1_d.rearrange("(m p) one -> p (m one)",
                                                  p=P))
        b2_sb = singles.tile([P, 1], f32)
        nc.scalar.dma_start(b2_sb, b2_d)
        b3_sb = singles.tile([D_OUT, 1], f32)
        nc.scalar.dma_start(b3_sb, b3_d)
        ident = singles.tile([P, P], f32)
        make_identity(nc, ident)
        identb = singles.tile([P, P], bf16)
        make_identity(nc, identb)

        # ---- main loop: masked attention scores + aggregation ----
        for rep in range(reps):
          accS = []
          with tc.tile_pool(name=f"accp{rep}", bufs=n_half,
                            space="PSUM") as accp:
            acc = [accp.tile([D_AUG, 512], f32, tag="acc", name=f"acc{i}")
                   for i in range(n_half)]
            for sc in range(n_sc):
                if rep == 0 and sc in pre_adjm:
                    adjm = pre_adjm.pop(sc)
                else:
                    adjm = adjp.tile([P, sc_chunks, rows], bf16)
                    nc.sync.dma_start(adjm.rearrange("p a b -> p (a b)"),
                                      adjm_d[sc * P:(sc + 1) * P, :])
                t1 = t1p.tile([P, sc_chunks, rows], bf16)
                for cc in range(sc_chunks):
                    jc = sc * sc_chunks + cc
                    eng = nc.gpsimd if cc < gp_chunks else nc.vector
                    eng.tensor_scalar(
                        t1[:, cc, :], adjm[:, cc, :],
                        0.2, sJm_sb[:, jc:jc + 1], OP.mult, OP.add)
                q = qp.tile([P, sc_chunks, rows], bf16)
                qf = q.rearrange("p a b -> p (a b)")
                nc.vector.tensor_tensor(
                    qf, adjm.rearrange("p a b -> p (a b)"),
                    t1.rearrange("p a b -> p (a b)"), OP.max)
                nc.scalar.activation(qf, qf, AF.Exp)
                for cc in range(sc_chunks):
                    jc = sc * sc_chunks + cc
                    for h in range(n_half):
                        nc.tensor.matmul(
                            acc[h][:, :],
                            lhsT=whu_sb[:, jc, :],
                            rhs=q[:, cc, h * 512:(h + 1) * 512],
                            start=(jc == 0),
                            stop=(jc == n_chunk - 1))

            # evacuate accumulators to SBUF so PSUM banks free up
            for h in range(n_half):
                aS = hp.tile([D_AUG, 512], f32, tag="accS", bufs=n_half)
                nc.vector.tensor_copy(aS, acc[h])
                accS.append(aS)

          # ---- epilogue: transpose to natural layout, LN stats, MLP ----
          with tc.tile_pool(name=f"mlpp{rep}", bufs=1, space="PSUM") as mlpp:
            for h in range(n_half):
                accn = hp.tile([P, 4, D_AUG], f32, tag="accn", bufs=2)
                for k in range(4):
                    tp = mlpp.tile([P, D_AUG], f32, tag="tp", bufs=2)
                    nc.tensor.transpose(tp, accS[h][:, k * P:(k + 1) * P],
                                        ident[0:D_AUG, 0:D_AUG])
                    nc.vector.tensor_copy(accn[:, k, :], tp)
                num = accn[:, :, 0:D_H]                    # [128, 4, 48]
                Dn = accn[:, :, 64:65].rearrange("p a one -> p (a one)")
                ssum = hp.tile([P, 4], f32, tag="ssum")
                nc.vector.tensor_reduce(ssum, num, axis=AX.X, op=OP.add)
                sqt = hp.tile([P, 4, D_H], f32, tag="sqt")
                nc.vector.tensor_tensor(sqt, num, num, OP.mult)
                ssq = hp.tile([P, 4], f32, tag="ssq")
                nc.vector.tensor_reduce(ssq, sqt, axis=AX.X, op=OP.add)
                mu = hp.tile([P, 4], f32, tag="mu")
                nc.scalar.activation(mu, ssum, AF.Copy, scale=1.0 / D_H)
                var = hp.tile([P, 4], f32, tag="var")
                nc.scalar.activation(var, ssq, AF.Copy, scale=1.0 / D_H)
                musq = hp.tile([P, 4], f32, tag="musq")
                nc.vector.tensor_tensor(musq, mu, mu, OP.mult)
                nc.vector.tensor_tensor(var, var, musq, OP.subtract)
                dsq = hp.tile([P, 4], f32, tag="dsq")
                nc.vector.tensor_tensor(dsq, Dn, Dn, OP.mult)
                nc.vector.tensor_scalar(dsq, dsq, EPS, None, OP.mult)
                nc.vector.tensor_tensor(var, var, dsq, OP.add)
                std = hp.tile([P, 4], f32, tag="std")
                nc.scalar.activation(std, var, AF.Sqrt)
                f = hp.tile([P, 4], f32, tag="f")
                nc.vector.reciprocal(f, std)
                hn = hp.tile([P, 4, D_H], bf16, tag="hn")
                for k in range(4):
                    nc.vector.tensor_scalar(
                        hn[:, k, :], num[:, k, :],
                        mu[:, k:k + 1], f[:, k:k + 1],
                        OP.subtract, OP.mult)
                hT = hp.tile([D_H, 512], bf16, tag="hT", bufs=2)
                for k in range(4):
                    tph = mlpp.tile([D_H, P], bf16, tag="tph", bufs=2)
                    nc.tensor.transpose(tph, hn[:, k, :], identb)
                    nc.vector.tensor_copy(hT[:, k * P:(k + 1) * P], tph)
                # MLP head 48 -> 256 -> 128 -> 32 (bf16 weights)
                h1 = hp.tile([P, 2, 512], bf16, tag="h1")
                for m in range(2):
                    m1 = mlpp.tile([P, 512], f32, tag="m1", bufs=2)
                    nc.tensor.matmul(m1, lhsT=w1g_sb[:, m * P:(m + 1) * P],
                                     rhs=hT, start=True, stop=True)
                    nc.scalar.activation(h1[:, m, :], m1, AF.Relu,
                                         bias=b1_sb[:, m:m + 1])
                m2 = mlpp.tile([P, 512], f32, tag="m2")
                for m in range(2):
                    nc.tensor.matmul(m2, lhsT=w2t_sb[:, m, :],
                                     rhs=h1[:, m, :],
                                     start=(m == 0), stop=(m == 1))
                h2 = hp.tile([P, 512], bf16, tag="h2")
                nc.scalar.activation(h2, m2, AF.Relu, bias=b2_sb)
                m3 = mlpp.tile([D_OUT, 512], f32, tag="m3")
                nc.tensor.matmul(m3, lhsT=w3t_sb, rhs=h2,
                                 start=True, stop=True)
                h3 = hp.tile([D_OUT, 512], f32, tag="h3")
                nc.scalar.activation(h3, m3, AF.Identity, bias=b3_sb)
                nc.sync.dma_start(out_d[h], h3)

    nc.compile()
    return nc


def host_prep(x, adj, W_gat, a, gamma, beta, W1, b1, W2, b2, W3, b3,
              num_cores=N_CORES):
    bf16 = ml_dtypes.bfloat16
    n = x.shape[0]
    rows = n // num_cores
    n_chunk = n // P
    n_sc = max(1, n_chunk // SC_CHUNKS)
    sc_chunks = n_chunk // n_sc
    Wh = (x @ W_gat.T).astype(np.float32)
    s = (Wh @ a.T).astype(np.float32).ravel()
    u = np.exp(s)
    whu = np.zeros((n, D_AUG), np.float32)
    whu[:, :D_H] = Wh * u[:, None]
    whu[:, 64] = u
    whu_r = np.ascontiguousarray(
        whu.reshape(n_chunk, P, D_AUG).transpose(1, 0, 2)
        .reshape(P, n_chunk * D_AUG)).astype(bf16)
    sJm = np.ascontiguousarray((-0.8 * s).reshape(n_chunk, P).T)
    s_bf = s.astype(bf16).astype(np.float32)
    # fold LayerNorm gamma/beta into the first MLP layer
    W1g = (W1 * gamma[None, :]).astype(np.float32)
    b1g = (b1 + W1 @ beta).astype(np.float32)
    adjT = np.ascontiguousarray(adj.T)  # adjT[j, i] = adj[i, j]
    in_maps = []
    for c in range(num_cores):
        r = slice(c * rows, (c + 1) * rows)
        M = np.where(adjT[:, r] > 0, s_bf[r][None, :],
                     np.float32(MASK_VAL)).astype(bf16)
        M = np.ascontiguousarray(
            M.reshape(n_sc, sc_chunks, P, rows).transpose(0, 2, 1, 3)
            .reshape(n_sc * P, sc_chunks * rows))
        in_maps.append({
            "adjm": M,
            "whu": whu_r,
            "sJm": sJm,
            "w1g": np.ascontiguousarray(W1g.T).astype(bf16),
            "b1": np.ascontiguousarray(b1g[:, None]).astype(np.float32),
            "w2t": np.ascontiguousarray(W2.T).astype(bf16),
            "b2": np.ascontiguousarray(b2[:, None]).astype(np.float32),
            "w3t": np.ascontiguousarray(W3.T).astype(bf16),
            "b3": np.ascontiguousarray(b3[:, None]).astype(np.float32),
        })
    return in_maps


def unpack_out(out_t):
    """[n_half, D_OUT, 512] transposed output -> [rows, D_OUT]."""
    return np.ascontiguousarray(
        np.transpose(np.asarray(out_t), (0, 2, 1)).reshape(-1, D_OUT))


_NC_CACHE = {}


def kernel(x, adj, W_gat, a, gamma, beta, W1, b1, W2, b2, W3, b3,
           trace=False):
    from concourse.bass_utils import run_bass_kernel_spmd

    args = [np.asarray(t) for t in
            (x, adj, W_gat, a, gamma, beta, W1, b1, W2, b2, W3, b3)]
    in_maps = host_prep(*args)
    if "nc" not in _NC_CACHE:
        _NC_CACHE["nc"] = build_nc()
    nc = _NC_CACHE["nc"]
    res = run_bass_kernel_spmd(nc, in_maps, list(range(N_CORES)), trace=trace)
    out = np.concatenate([unpack_out(r["out"]) for r in res.results], axis=0)
    if trace:
        kernel.last_results = res
    return out.astype(np.float32)


# revision 19
# speedup vs baseline: 1.0661x; 1.0661x over previous
"""Trainium2 Bass kernel for nn_MetaRL_LightGAT_BiACT (GAT + LayerNorm + MLP).

Strategy (8 NeuronCores, row-sharded, transposed layout [j_part, i_free]):
  - Each core owns 1024 of the 8192 output rows (node dim N=i); the full
    j dim (8192) is reduced on-chip via PSUM accumulation.
  - Host precomputes the tiny GAT projection Wh = x @ W_gat.T and scores
    s = Wh @ a.T (0.15% of FLOPs), and marshals adj into a single bf16
    tensor  adjm[j, i] = adj[i, j] ? s_i : -60   (pre-transposed and
    pre-tiled so each superchunk is one contiguous [128, sc*1024] DMA slab).
  - Identity used on device, per element (w = adjm):
        adj * exp(prelu(s_i + s_j))
      = exp(max(w, 0.2*w - 0.8*s_j) + s_j)            (w = s_i on edges)
      = exp(max(w, 0.2*w - 0.8*s_j)) * e^{s_j}
    with e^{s_j} folded into the matmul weights WhU[j,:] = e^{s_j}*Wh[j,:]
    (and the softmax-denominator ones column becomes e^{s_j}).
    Non-edges (w = -60) give exp(<= -11) ~ 0, i.e. the mask.
  - Device main loop per superchunk (512 j's):
      DVE  tensor_scalar (bf16): t1 = (w - 4*s_j) * 0.2    (per 128-chunk)
      DVE  tensor_tensor (bf16 2x): q = max(w, t1)
      ACT  activation    Exp       : q = exp(q)            (whole slab)
      PE   matmul bf16: acc[65, i] += WhU_chunk^T @ q_chunk  (PSUM accum,
           col 64 of WhU is e^{s_j} -> softmax denominator D for free)
    DVE and ACT are the co-bottlenecks; to balance them, a few chunks per
    run take an ACT-only route instead:  q = Prelu(w + s_j)  (bias is
    per-partition s_j), whose exp then equals exp(prelu(s_i+s_j)) WITHOUT
    the e^{s_j} factor -- host skips the u-fold in whu for those j-chunks.
  - Epilogue (both halves merged into wide ops): LayerNorm is invariant
    to the positive per-column scale 1/D except through eps:
        (h'-mu)/sqrt(var+eps) = (num - mu_num) / sqrt(var_num + eps*D^2)
    so the attention normalization is never applied explicitly. The
    [65, 1024] accumulator pair is PE-transposed to natural [i_part, d]
    layout where all per-i stats (sums, sqrt, reciprocal) vectorize
    across 128 lanes, gamma/beta are folded into W1/b1 on host, and the
    48->256->128->32 MLP runs in bf16 after transposing back. The final
    [32, 1024] result is DMA'd out transposed; the host un-transposes.
"""

import sys

if "/opt/trn_rl_repo" not in sys.path:
    sys.path.insert(0, "/opt/trn_rl_repo")

import numpy as np
import ml_dtypes

N = 8192
D_IN = 128
D_H = 48
D_AUG = 65  # WhU cols 0-47, zeros 48-63, e^{s_j} (or 1) col at 64
D_OUT = 32
N_CORES = 8
ROWS = N // N_CORES          # 1024 rows per core
P = 128                      # partitions
SC_CHUNKS = 4                # j-chunks per superchunk
MASK_VAL = -60.0
EPS = 1e-5
ACTP_EVERY = 2               # every ACTP_EVERY-th superchunk routes its last
ACTP_OFF = 1                 # chunk via ACT-Prelu (0 disables)


def actp_chunks(n_chunk):
    """Set of j-chunk indices that take the ACT-Prelu route."""
    n_sc = max(1, n_chunk // SC_CHUNKS)
    sc_chunks = n_chunk // n_sc
    out = set()
    if ACTP_EVERY:
        for sc in range(ACTP_OFF, n_sc, ACTP_EVERY):
            out.add(sc * sc_chunks + (sc_chunks - 1))
    return out


def build_nc(num_cores=N_CORES, rows=ROWS, n=N, reps=1,
             prefetch=2, adj_bufs=4, t1_bufs=3, q_bufs=3):
    import concourse.bass as bass
    import concourse.mybir as mybir
    import concourse.tile as tile
    from concourse import bacc
    from concourse.masks import make_identity
    from contextlib import ExitStack

    f32 = mybir.dt.float32
    bf16 = mybir.dt.bfloat16
    AF = mybir.ActivationFunctionType
    OP = mybir.AluOpType
    AX = mybir.AxisListType

    n_chunk = n // P
    n_sc = max(1, n_chunk // SC_CHUNKS)
    sc_chunks = n_chunk // n_sc
    n_half = rows // 512
    actp = actp_chunks(n_chunk)

    nc = bacc.Bacc("TRN2", target_bir_lowering=False, debug=False,
                   num_devices=num_cores)

    adjm_d = nc.dram_tensor("adjm", [n_sc * P, sc_chunks * rows], bf16,
                            kind="ExternalInput").ap()
    whu_d = nc.dram_tensor("whu", [P, n_chunk * D_AUG], bf16,
                           kind="ExternalInput").ap()
    sJm_d = nc.dram_tensor("sJm", [P, n_chunk], f32, kind="ExternalInput").ap()
    sJp_d = nc.dram_tensor("sJp", [P, n_chunk], f32, kind="ExternalInput").ap()
    w1g_d = nc.dram_tensor("w1g", [D_H, 256], bf16, kind="ExternalInput").ap()
    b1_d = nc.dram_tensor("b1", [256, 1], f32, kind="ExternalInput").ap()
    w2t_d = nc.dram_tensor("w2t", [256, 128], bf16, kind="ExternalInput").ap()
    b2_d = nc.dram_tensor("b2", [128, 1], f32, kind="ExternalInput").ap()
    w3t_d = nc.dram_tensor("w3t", [128, D_OUT], bf16, kind="ExternalInput").ap()
    b3_d = nc.dram_tensor("b3", [D_OUT, 1], f32, kind="ExternalInput").ap()
    out_d = nc.dram_tensor("out", [D_OUT, rows], f32,
                           kind="ExternalOutput").ap()

    with ExitStack() as ctx:
        tc = ctx.enter_context(tile.TileContext(nc))
        singles = ctx.enter_context(tc.tile_pool(name="singles", bufs=1))
        adjp = ctx.enter_context(tc.tile_pool(name="adjp", bufs=adj_bufs))
        t1p = ctx.enter_context(tc.tile_pool(name="t1p", bufs=t1_bufs))
        qp = ctx.enter_context(tc.tile_pool(name="qp", bufs=q_bufs))
        hp = ctx.enter_context(tc.tile_pool(name="hp", bufs=2))

        # sJ tables first (needed by the very first TS), then prefetch the
        # first adjm slabs on the Sync queue; first slab in two halves so
        # the pipeline can start after 512KB instead of 1MB.
        sJm_sb = singles.tile([P, n_chunk], f32)
        nc.scalar.dma_start(sJm_sb, sJm_d)
        sJp_sb = singles.tile([P, n_chunk], f32)
        nc.scalar.dma_start(sJp_sb, sJp_d)
        pre_adjm = {}
        for sc in range(min(prefetch, n_sc)):
            adjm = adjp.tile([P, sc_chunks, rows], bf16, name=f"adjm{sc}")
            fl = adjm.rearrange("p a b -> p (a b)")
            w = sc_chunks * rows
            if sc == 0:
                nc.sync.dma_start(fl[:, 0:w // 2],
                                  adjm_d[0:P, 0:w // 2])
                nc.sync.dma_start(fl[:, w // 2:w],
                                  adjm_d[0:P, w // 2:w])
            else:
                nc.sync.dma_start(fl, adjm_d[sc * P:(sc + 1) * P, :])
            pre_adjm[sc] = adjm

        # remaining resident small tensors (scalar HWDGE queue)
        whu_sb = singles.tile([P, n_chunk, D_AUG], bf16)
        nc.scalar.dma_start(whu_sb, whu_d.rearrange("p (c d) -> p c d",
                                                    d=D_AUG))
        w1g_sb = singles.tile([D_H, 256], bf16)
        nc.scalar.dma_start(w1g_sb, w1g_d)
        w2t_sb = singles.tile([P, 2, 128], bf16)
        nc.scalar.dma_start(w2t_sb, w2t_d.rearrange("(m p) k -> p m k", p=P))
        w3t_sb = singles.tile([P, D_OUT], bf16)
        nc.scalar.dma_start(w3t_sb, w3t_d)
        b1_sb = singles.tile([P, 2], f32)
        nc.scalar.dma_start(b1_sb, b1_d.rearrange("(m p) one -> p (m one)",
                                                  p=P))
        b2_sb = singles.tile([P, 1], f32)
        nc.scalar.dma_start(b2_sb, b2_d)
        b3_sb = singles.tile([D_OUT, 1], f32)
        nc.scalar.dma_start(b3_sb, b3_d)
        ident = singles.tile([P, P], f32)
        make_identity(nc, ident)
        identb = singles.tile([P, P], bf16)
        make_identity(nc, identb)

        # ---- main loop: masked attention scores + aggregation ----
        for rep in range(reps):
          accS = []
          with tc.tile_pool(name=f"accp{rep}", bufs=n_half,
                            space="PSUM") as accp:
            acc = [accp.tile([D_AUG, 512], f32, tag="acc", name=f"acc{i}")
                   for i in range(n_half)]
            for sc in range(n_sc):
                if rep == 0 and sc in pre_adjm:
                    adjm = pre_adjm.pop(sc)
                else:
                    adjm = adjp.tile([P, sc_chunks, rows], bf16)
                    nc.sync.dma_start(adjm.rearrange("p a b -> p (a b)"),
                                      adjm_d[sc * P:(sc + 1) * P, :])
                t1 = t1p.tile([P, sc_chunks, rows], bf16)
                q = qp.tile([P, sc_chunks, rows], bf16)
                n_dve = sc_chunks
                for cc in range(sc_chunks):
                    jc = sc * sc_chunks + cc
                    if jc in actp:
                        # ACT route: q = prelu(w + s_j); exp comes below.
                        # (host left whu un-u-folded for this j-chunk)
                        nc.scalar.activation(
                            q[:, cc, :], adjm[:, cc, :], AF.Prelu,
                            bias=sJp_sb[:, jc:jc + 1], alpha=0.2)
                        n_dve = cc  # actp chunk is always last in the sc
                    else:
                        nc.vector.tensor_scalar(
                            t1[:, cc, :], adjm[:, cc, :],
                            sJm_sb[:, jc:jc + 1], 0.2, OP.add, OP.mult)
                nc.vector.tensor_tensor(
                    q[:, 0:n_dve, :].rearrange("p a b -> p (a b)"),
                    adjm[:, 0:n_dve, :].rearrange("p a b -> p (a b)"),
                    t1[:, 0:n_dve, :].rearrange("p a b -> p (a b)"), OP.max)
                qf = q.rearrange("p a b -> p (a b)")
                nc.scalar.activation(qf, qf, AF.Exp)
                for cc in range(sc_chunks):
                    jc = sc * sc_chunks + cc
                    for h in range(n_half):
                        nc.tensor.matmul(
                            acc[h][:, :],
                            lhsT=whu_sb[:, jc, :],
                            rhs=q[:, cc, h * 512:(h + 1) * 512],
                            start=(jc == 0),
                            stop=(jc == n_chunk - 1))

            # evacuate accumulators to SBUF so PSUM banks free up
            for h in range(n_half):
                aS = hp.tile([D_AUG, 512], f32, tag="accS", bufs=n_half)
                nc.vector.tensor_copy(aS, acc[h])
                accS.append(aS)

          # ---- epilogue: transpose to natural layout, LN stats, MLP ----
          with tc.tile_pool(name=f"mlpp{rep}", bufs=1, space="PSUM") as mlpp:
            nblk = n_half * 4
            # prefetch the sqrt activation table while transposes run
            dummy = hp.tile([1, 4], f32, tag="dummy")
            nc.vector.memset(dummy, 1.0)
            nc.scalar.activation(dummy, dummy, AF.Sqrt)
            accn = hp.tile([P, nblk, D_AUG], f32, tag="accn")
            for h in range(n_half):
                for k in range(4):
                    tp = mlpp.tile([P, D_AUG], f32, tag="tp")
                    nc.tensor.transpose(tp, accS[h][:, k * P:(k + 1) * P],
                                        ident[0:D_AUG, 0:D_AUG])
                    nc.vector.tensor_copy(accn[:, h * 4 + k, :], tp)
            num = accn[:, :, 0:D_H]                    # [128, nblk, 48]
            Dn = accn[:, :, 64:65].rearrange("p a one -> p (a one)")
            ssum = hp.tile([P, nblk], f32, tag="ssum")
            nc.vector.tensor_reduce(ssum, num, axis=AX.X, op=OP.add)
            sqt = hp.tile([P, nblk, D_H], f32, tag="sqt")
            nc.vector.tensor_tensor(sqt, num, num, OP.mult)
            ssq = hp.tile([P, nblk], f32, tag="ssq")
            nc.vector.tensor_reduce(ssq, sqt, axis=AX.X, op=OP.add)
            mu = hp.tile([P, nblk], f32, tag="mu")
            nc.scalar.activation(mu, ssum, AF.Copy, scale=1.0 / D_H)
            var = hp.tile([P, nblk], f32, tag="var")
            nc.scalar.activation(var, ssq, AF.Copy, scale=1.0 / D_H)
            musq = hp.tile([P, nblk], f32, tag="musq")
            nc.vector.tensor_tensor(musq, mu, mu, OP.mult)
            nc.vector.tensor_tensor(var, var, musq, OP.subtract)
            dsq = hp.tile([P, nblk], f32, tag="dsq")
            nc.vector.tensor_tensor(dsq, Dn, Dn, OP.mult)
            nc.vector.tensor_scalar(dsq, dsq, EPS, None, OP.mult)
            nc.vector.tensor_tensor(var, var, dsq, OP.add)
            std = hp.tile([P, nblk], f32, tag="std")
            nc.scalar.activation(std, var, AF.Sqrt)
            f = hp.tile([P, nblk], f32, tag="f")
            nc.vector.reciprocal(f, std)
            hn = hp.tile([P, nblk, D_H], bf16, tag="hn")
            for k in range(nblk):
                nc.vector.tensor_scalar(
                    hn[:, k, :], num[:, k, :],
                    mu[:, k:k + 1], f[:, k:k + 1],
                    OP.subtract, OP.mult)
            hT = hp.tile([D_H, rows], bf16, tag="hT")
            for k in range(nblk):
                tph = mlpp.tile([D_H, P], bf16, tag="tph")
                nc.tensor.transpose(tph, hn[:, k, :], identb)
                nc.vector.tensor_copy(hT[:, k * P:(k + 1) * P], tph)
            # MLP head 48 -> 256 -> 128 -> 32 (bf16, both halves wide)
            h1 = hp.tile([P, 2, rows], bf16, tag="h1")
            for m in range(2):
                m1 = mlpp.tile([P, n_half, 512], f32, tag="m1")
                for h in range(n_half):
                    nc.tensor.matmul(m1[:, h, :],
                                     lhsT=w1g_sb[:, m * P:(m + 1) * P],
                                     rhs=hT[:, h * 512:(h + 1) * 512],
                                     start=True, stop=True)
                nc.scalar.activation(h1[:, m, :],
                                     m1.rearrange("p a b -> p (a b)"),
                                     AF.Relu, bias=b1_sb[:, m:m + 1])
            m2 = mlpp.tile([P, n_half, 512], f32, tag="m2")
            for h in range(n_half):
                for m in range(2):
                    nc.tensor.matmul(m2[:, h, :], lhsT=w2t_sb[:, m, :],
                                     rhs=h1[:, m, h * 512:(h + 1) * 512],
                                     start=(m == 0), stop=(m == 1))
            h2 = hp.tile([P, rows], bf16, tag="h2")
            nc.scalar.activation(h2, m2.rearrange("p a b -> p (a b)"),
                                 AF.Relu, bias=b2_sb)
            m3 = [mlpp.tile([D_OUT, 512], f32, tag="m3", bufs=2,
                            name=f"m3_{hh}")
                  for hh in range(n_half)]
            for h in range(n_half):
                nc.tensor.matmul(m3[h], lhsT=w3t_sb,
                                 rhs=h2[:, h * 512:(h + 1) * 512],
                                 start=True, stop=True)
            h3 = hp.tile([D_OUT, rows], f32, tag="h3")
            for h in range(n_half):
                nc.scalar.activation(h3[:, h * 512:(h + 1) * 512], m3[h],
                                     AF.Identity, bias=b3_sb)
            nc.sync.dma_start(out_d, h3)

    nc.compile()
    return nc


def host_prep(x, adj, W_gat, a, gamma, beta, W1, b1, W2, b2, W3, b3,
              num_cores=N_CORES):
    bf16 = ml_dtypes.bfloat16
    n = x.shape[0]
    rows = n // num_cores
    n_chunk = n // P
    n_sc = max(1, n_chunk // SC_CHUNKS)
    sc_chunks = n_chunk // n_sc
    Wh = (x @ W_gat.T).astype(np.float32)
    s = (Wh @ a.T).astype(np.float32).ravel()
    u = np.exp(s)
    # chunks on the ACT-Prelu route compute exp(prelu(s_i+s_j)) directly,
    # so their whu rows must NOT carry the e^{s_j} fold
    uf = u.copy()
    for jc in actp_chunks(n_chunk):
        uf[jc * P:(jc + 1) * P] = 1.0
    whu = np.zeros((n, D_AUG), np.float32)
    whu[:, :D_H] = Wh * uf[:, None]
    whu[:, 64] = uf
    whu_r = np.ascontiguousarray(
        whu.reshape(n_chunk, P, D_AUG).transpose(1, 0, 2)
        .reshape(P, n_chunk * D_AUG)).astype(bf16)
    sJm = np.ascontiguousarray((-4.0 * s).reshape(n_chunk, P).T)
    sJp = np.ascontiguousarray(s.reshape(n_chunk, P).T)
    s_bf = s.astype(bf16).astype(np.float32)
    # fold LayerNorm gamma/beta into the first MLP layer
    W1g = (W1 * gamma[None, :]).astype(np.float32)
    b1g = (b1 + W1 @ beta).astype(np.float32)
    adjT = np.ascontiguousarray(adj.T)  # adjT[j, i] = adj[i, j]
    in_maps = []
    for c in range(num_cores):
        r = slice(c * rows, (c + 1) * rows)
        M = np.where(adjT[:, r] > 0, s_bf[r][None, :],
                     np.float32(MASK_VAL)).astype(bf16)
        M = np.ascontiguousarray(
            M.reshape(n_sc, sc_chunks, P, rows).transpose(0, 2, 1, 3)
            .reshape(n_sc * P, sc_chunks * rows))
        in_maps.append({
            "adjm": M,
            "whu": whu_r,
            "sJm": sJm,
            "sJp": sJp,
            "w1g": np.ascontiguousarray(W1g.T).astype(bf16),
            "b1": np.ascontiguousarray(b1g[:, None]).astype(np.float32),
            "w2t": np.ascontiguousarray(W2.T).astype(bf16),
            "b2": np.ascontiguousarray(b2[:, None]).astype(np.float32),
            "w3t": np.ascontiguousarray(W3.T).astype(bf16),
            "b3": np.ascontiguousarray(b3[:, None]).astype(np.float32),
        })
    return in_maps


def unpack_out(out_t):
    """[D_OUT, rows] transposed output -> [rows, D_OUT]."""
    return np.ascontiguousarray(np.asarray(out_t).T)


_NC_CACHE = {}


def kernel(x, adj, W_gat, a, gamma, beta, W1, b1, W2, b2, W3, b3,
           trace=False):
    from concourse.bass_utils import run_bass_kernel_spmd

    args = [np.asarray(t) for t in
            (x, adj, W_gat, a, gamma, beta, W1, b1, W2, b2, W3, b3)]
    in_maps = host_prep(*args)
    if "nc" not in _NC_CACHE:
        _NC_CACHE["nc"] = build_nc()
    nc = _NC_CACHE["nc"]
    res = run_bass_kernel_spmd(nc, in_maps, list(range(N_CORES)), trace=trace)
    out = np.concatenate([unpack_out(r["out"]) for r in res.results], axis=0)
    if trace:
        kernel.last_results = res
    return out.astype(np.float32)


# revision 20
# speedup vs baseline: 1.1257x; 1.0559x over previous
"""Trainium2 Bass kernel for nn_MetaRL_LightGAT_BiACT (GAT + LayerNorm + MLP).

Strategy (8 NeuronCores, row-sharded, transposed layout [j_part, i_free]):
  - Each core owns 1024 of the 8192 output rows (node dim N=i); the full
    j dim (8192) is reduced on-chip via PSUM accumulation.
  - Host precomputes the tiny GAT projection Wh = x @ W_gat.T and scores
    s = Wh @ a.T (0.15% of FLOPs), and marshals adj into a single bf16
    tensor  adjm[j, i] = adj[i, j] ? s_i : -60   (pre-transposed and
    pre-tiled so each superchunk is one contiguous [128, sc*1024] DMA slab).
  - Identity used on device, per element (w = adjm):
        adj * exp(prelu(s_i + s_j))
      = exp(max(w, 0.2*w - 0.8*s_j) + s_j)            (w = s_i on edges)
      = exp(max(w, 0.2*w - 0.8*s_j)) * e^{s_j}
    with e^{s_j} folded into the matmul weights WhU[j,:] = e^{s_j}*Wh[j,:]
    (and the softmax-denominator ones column becomes e^{s_j}).
    Non-edges (w = -60) give exp(<= -11) ~ 0, i.e. the mask.
  - Device main loop per superchunk (512 j's):
      DVE  tensor_scalar (bf16): t1 = (w - 4*s_j) * 0.2    (per 128-chunk)
      DVE  tensor_tensor (bf16 2x): q = max(w, t1)
      ACT  activation    Exp       : q = exp(q)            (whole slab)
      PE   matmul bf16: acc[65, i] += WhU_chunk^T @ q_chunk  (PSUM accum,
           col 64 of WhU is e^{s_j} -> softmax denominator D for free)
    DVE and ACT are the co-bottlenecks; to balance them, a few chunks per
    run take an ACT-only route instead:  q = Prelu(w + s_j)  (bias is
    per-partition s_j), whose exp then equals exp(prelu(s_i+s_j)) WITHOUT
    the e^{s_j} factor -- host skips the u-fold in whu for those j-chunks.
  - Epilogue (both halves merged into wide ops): LayerNorm is invariant
    to the positive per-column scale 1/D except through eps:
        (h'-mu)/sqrt(var+eps) = (num - mu_num) / sqrt(var_num + eps*D^2)
    so the attention normalization is never applied explicitly. The
    [65, 1024] accumulator pair is PE-transposed to natural [i_part, d]
    layout where all per-i stats (sums, sqrt, reciprocal) vectorize
    across 128 lanes, gamma/beta are folded into W1/b1 on host, and the
    48->256->128->32 MLP runs in bf16 after transposing back. The final
    [32, 1024] result is DMA'd out transposed; the host un-transposes.
"""

import sys

if "/opt/trn_rl_repo" not in sys.path:
    sys.path.insert(0, "/opt/trn_rl_repo")

import numpy as np
import ml_dtypes

N = 8192
D_IN = 128
D_H = 48
D_AUG = 65  # WhU cols 0-47, zeros 48-63, e^{s_j} (or 1) col at 64
D_OUT = 32
N_CORES = 8
ROWS = N // N_CORES          # 1024 rows per core
P = 128                      # partitions
SC_CHUNKS = 4                # j-chunks per superchunk
MASK_VAL = -60.0
EPS = 1e-5
ACTP_EVERY = 4               # every ACTP_EVERY-th superchunk routes its last
ACTP_OFF = 1                 # chunk via ACT-Prelu (0 disables)


def actp_chunks(n_chunk):
    """Set of j-chunk indices that take the ACT-Prelu route."""
    n_sc = max(1, n_chunk // SC_CHUNKS)
    sc_chunks = n_chunk // n_sc
    out = set()
    if ACTP_EVERY:
        for sc in range(ACTP_OFF, n_sc, ACTP_EVERY):
            out.add(sc * sc_chunks + (sc_chunks - 1))
    return out


def build_nc(num_cores=N_CORES, rows=ROWS, n=N, reps=1,
             prefetch=2, adj_bufs=4, t1_bufs=3, q_bufs=3):
    import concourse.bass as bass
    import concourse.mybir as mybir
    import concourse.tile as tile
    from concourse import bacc
    from concourse.masks import make_identity
    from contextlib import ExitStack

    f32 = mybir.dt.float32
    bf16 = mybir.dt.bfloat16
    AF = mybir.ActivationFunctionType
    OP = mybir.AluOpType
    AX = mybir.AxisListType

    n_chunk = n // P
    n_sc = max(1, n_chunk // SC_CHUNKS)
    sc_chunks = n_chunk // n_sc
    n_half = rows // 512
    actp = actp_chunks(n_chunk)

    nc = bacc.Bacc("TRN2", target_bir_lowering=False, debug=False,
                   num_devices=num_cores)

    adjm_d = nc.dram_tensor("adjm", [n_sc * P, sc_chunks * rows], bf16,
                            kind="ExternalInput").ap()
    whu_d = nc.dram_tensor("whu", [P, n_chunk * D_AUG], bf16,
                           kind="ExternalInput").ap()
    sJm_d = nc.dram_tensor("sJm", [P, n_chunk], f32, kind="ExternalInput").ap()
    sJp_d = nc.dram_tensor("sJp", [P, n_chunk], f32, kind="ExternalInput").ap()
    w1g_d = nc.dram_tensor("w1g", [D_H, 256], bf16, kind="ExternalInput").ap()
    b1_d = nc.dram_tensor("b1", [256, 1], f32, kind="ExternalInput").ap()
    w2t_d = nc.dram_tensor("w2t", [256, 128], bf16, kind="ExternalInput").ap()
    b2_d = nc.dram_tensor("b2", [128, 1], f32, kind="ExternalInput").ap()
    w3t_d = nc.dram_tensor("w3t", [128, D_OUT], bf16, kind="ExternalInput").ap()
    b3_d = nc.dram_tensor("b3", [D_OUT, 1], f32, kind="ExternalInput").ap()
    out_d = nc.dram_tensor("out", [D_OUT, rows], f32,
                           kind="ExternalOutput").ap()

    with ExitStack() as ctx:
        tc = ctx.enter_context(tile.TileContext(nc))
        singles = ctx.enter_context(tc.tile_pool(name="singles", bufs=1))
        adjp = ctx.enter_context(tc.tile_pool(name="adjp", bufs=adj_bufs))
        t1p = ctx.enter_context(tc.tile_pool(name="t1p", bufs=t1_bufs))
        qp = ctx.enter_context(tc.tile_pool(name="qp", bufs=q_bufs))
        hp = ctx.enter_context(tc.tile_pool(name="hp", bufs=2))

        # sJ tables first (needed by the very first TS), then prefetch the
        # first adjm slabs on the Sync queue; first slab in two halves so
        # the pipeline can start after 512KB instead of 1MB.
        sJm_sb = singles.tile([P, n_chunk], f32)
        nc.scalar.dma_start(sJm_sb, sJm_d)
        sJp_sb = singles.tile([P, n_chunk], f32)
        nc.scalar.dma_start(sJp_sb, sJp_d)
        pre_adjm = {}
        for sc in range(min(prefetch, n_sc)):
            adjm = adjp.tile([P, sc_chunks, rows], bf16, name=f"adjm{sc}")
            fl = adjm.rearrange("p a b -> p (a b)")
            w = sc_chunks * rows
            if sc == 0:
                nc.sync.dma_start(fl[:, 0:w // 2],
                                  adjm_d[0:P, 0:w // 2])
                nc.sync.dma_start(fl[:, w // 2:w],
                                  adjm_d[0:P, w // 2:w])
            else:
                nc.sync.dma_start(fl, adjm_d[sc * P:(sc + 1) * P, :])
            pre_adjm[sc] = adjm

        # remaining resident small tensors (scalar HWDGE queue)
        whu_sb = singles.tile([P, n_chunk, D_AUG], bf16)
        nc.scalar.dma_start(whu_sb, whu_d.rearrange("p (c d) -> p c d",
                                                    d=D_AUG))
        w1g_sb = singles.tile([D_H, 256], bf16)
        nc.scalar.dma_start(w1g_sb, w1g_d)
        w2t_sb = singles.tile([P, 2, 128], bf16)
        nc.scalar.dma_start(w2t_sb, w2t_d.rearrange("(m p) k -> p m k", p=P))
        w3t_sb = singles.tile([P, D_OUT], bf16)
        nc.scalar.dma_start(w3t_sb, w3t_d)
        b1_sb = singles.tile([P, 2], f32)
        nc.scalar.dma_start(b1_sb, b1_d.rearrange("(m p) one -> p (m one)",
                                                  p=P))
        b2_sb = singles.tile([P, 1], f32)
        nc.scalar.dma_start(b2_sb, b2_d)
        b3_sb = singles.tile([D_OUT, 1], f32)
        nc.scalar.dma_start(b3_sb, b3_d)
        ident = singles.tile([P, P], f32)
        make_identity(nc, ident)
        identb = singles.tile([P, P], bf16)
        make_identity(nc, identb)

        # ---- main loop: masked attention scores + aggregation ----
        for rep in range(reps):
          accS = []
          with tc.tile_pool(name=f"accp{rep}", bufs=n_half,
                            space="PSUM") as accp:
            acc = [accp.tile([D_AUG, 512], f32, tag="acc", name=f"acc{i}")
                   for i in range(n_half)]
            for sc in range(n_sc):
                if rep == 0 and sc in pre_adjm:
                    adjm = pre_adjm.pop(sc)
                else:
                    adjm = adjp.tile([P, sc_chunks, rows], bf16)
                    nc.sync.dma_start(adjm.rearrange("p a b -> p (a b)"),
                                      adjm_d[sc * P:(sc + 1) * P, :])
                t1 = t1p.tile([P, sc_chunks, rows], bf16)
                q = qp.tile([P, sc_chunks, rows], bf16)
                n_dve = sc_chunks
                for cc in range(sc_chunks):
                    jc = sc * sc_chunks + cc
                    if jc in actp:
                        # ACT route: q = prelu(w + s_j); exp comes below.
                        # (host left whu un-u-folded for this j-chunk)
                        nc.scalar.activation(
                            q[:, cc, :], adjm[:, cc, :], AF.Prelu,
                            bias=sJp_sb[:, jc:jc + 1], alpha=0.2)
                        n_dve = cc  # actp chunk is always last in the sc
                    else:
                        nc.vector.tensor_scalar(
                            t1[:, cc, :], adjm[:, cc, :],
                            sJm_sb[:, jc:jc + 1], 0.2, OP.add, OP.mult)
                nc.vector.tensor_tensor(
                    q[:, 0:n_dve, :].rearrange("p a b -> p (a b)"),
                    adjm[:, 0:n_dve, :].rearrange("p a b -> p (a b)"),
                    t1[:, 0:n_dve, :].rearrange("p a b -> p (a b)"), OP.max)
                qf = q.rearrange("p a b -> p (a b)")
                nc.scalar.activation(qf, qf, AF.Exp)
                for cc in range(sc_chunks):
                    jc = sc * sc_chunks + cc
                    for h in range(n_half):
                        nc.tensor.matmul(
                            acc[h][:, :],
                            lhsT=whu_sb[:, jc, :],
                            rhs=q[:, cc, h * 512:(h + 1) * 512],
                            start=(jc == 0),
                            stop=(jc == n_chunk - 1))

            # evacuate accumulators to SBUF so PSUM banks free up
            # (split across DVE and ACT so the two copies overlap)
            for h in range(n_half):
                aS = hp.tile([D_AUG, 512], f32, tag="accS", bufs=n_half)
                if h % 2 == 0:
                    nc.vector.tensor_copy(aS, acc[h])
                else:
                    nc.scalar.activation(aS, acc[h], AF.Copy)
                accS.append(aS)

          # ---- epilogue: transpose to natural layout, LN stats, MLP ----
          with tc.tile_pool(name=f"mlpp{rep}", bufs=1, space="PSUM") as mlpp:
            nblk = n_half * 4
            # prefetch the sqrt activation table while transposes run
            # (source b2_sb is ready since startup - no DVE dependency)
            dummy = hp.tile([P, 1], f32, tag="dummy")
            nc.scalar.activation(dummy, b2_sb, AF.Sqrt)
            accn = hp.tile([P, nblk, D_AUG], f32, tag="accn")
            for h in range(n_half):
                tp = mlpp.tile([P, 4, D_AUG], f32, tag="tp")
                for k in range(4):
                    nc.tensor.transpose(tp[:, k, :],
                                        accS[h][:, k * P:(k + 1) * P],
                                        ident[0:D_AUG, 0:D_AUG])
                nc.vector.tensor_copy(accn[:, h * 4:h * 4 + 4, :], tp)
            num = accn[:, :, 0:D_H]                    # [128, nblk, 48]
            Dn = accn[:, :, 64:65].rearrange("p a one -> p (a one)")
            ssum = hp.tile([P, nblk], f32, tag="ssum")
            nc.vector.tensor_reduce(ssum, num, axis=AX.X, op=OP.add)
            sqt = hp.tile([P, nblk, D_H], f32, tag="sqt")
            nc.vector.tensor_tensor(sqt, num, num, OP.mult)
            ssq = hp.tile([P, nblk], f32, tag="ssq")
            nc.vector.tensor_reduce(ssq, sqt, axis=AX.X, op=OP.add)
            mu = hp.tile([P, nblk], f32, tag="mu")
            nc.scalar.activation(mu, ssum, AF.Copy, scale=1.0 / D_H)
            var = hp.tile([P, nblk], f32, tag="var")
            nc.scalar.activation(var, ssq, AF.Copy, scale=1.0 / D_H)
            musq = hp.tile([P, nblk], f32, tag="musq")
            nc.vector.tensor_tensor(musq, mu, mu, OP.mult)
            nc.vector.tensor_tensor(var, var, musq, OP.subtract)
            dsq = hp.tile([P, nblk], f32, tag="dsq")
            nc.vector.tensor_tensor(dsq, Dn, Dn, OP.mult)
            nc.vector.tensor_scalar(dsq, dsq, EPS, None, OP.mult)
            nc.vector.tensor_tensor(var, var, dsq, OP.add)
            std = hp.tile([P, nblk], f32, tag="std")
            nc.scalar.activation(std, var, AF.Sqrt)
            f = hp.tile([P, nblk], f32, tag="f")
            nc.vector.reciprocal(f, std)
            negmuf = hp.tile([P, nblk], f32, tag="negmuf")
            nc.vector.tensor_tensor(negmuf, mu, f, OP.mult)
            nc.vector.tensor_scalar(negmuf, negmuf, -1.0, None, OP.mult)
            hn = hp.tile([P, nblk, D_H], bf16, tag="hn")
            for k in range(nblk):
                # (num - mu) * f on ACT: f*num + (-mu*f), per-partition APs
                nc.scalar.activation(
                    hn[:, k, :], num[:, k, :], AF.Identity,
                    bias=negmuf[:, k:k + 1], scale=f[:, k:k + 1])
            hT = hp.tile([D_H, rows], bf16, tag="hT")
            for h in range(n_half):
                tph = mlpp.tile([D_H, 4, P], bf16, tag="tph")
                for k in range(4):
                    nc.tensor.transpose(tph[:, k, :], hn[:, h * 4 + k, :],
                                        identb)
                nc.vector.tensor_copy(
                    hT[:, h * 512:(h + 1) * 512],
                    tph.rearrange("p a b -> p (a b)"))
            # MLP head 48 -> 256 -> 128 -> 32 (bf16, both halves wide)
            h1 = hp.tile([P, 2, rows], bf16, tag="h1")
            for m in range(2):
                m1 = mlpp.tile([P, n_half, 512], f32, tag="m1")
                for h in range(n_half):
                    nc.tensor.matmul(m1[:, h, :],
                                     lhsT=w1g_sb[:, m * P:(m + 1) * P],
                                     rhs=hT[:, h * 512:(h + 1) * 512],
                                     start=True, stop=True)
                nc.scalar.activation(h1[:, m, :],
                                     m1.rearrange("p a b -> p (a b)"),
                                     AF.Relu, bias=b1_sb[:, m:m + 1])
            m2 = mlpp.tile([P, n_half, 512], f32, tag="m2")
            for h in range(n_half):
                for m in range(2):
                    nc.tensor.matmul(m2[:, h, :], lhsT=w2t_sb[:, m, :],
                                     rhs=h1[:, m, h * 512:(h + 1) * 512],
                                     start=(m == 0), stop=(m == 1))
            h2 = hp.tile([P, rows], bf16, tag="h2")
            nc.scalar.activation(h2, m2.rearrange("p a b -> p (a b)"),
                                 AF.Relu, bias=b2_sb)
            m3 = [mlpp.tile([D_OUT, 512], f32, tag="m3", bufs=2,
                            name=f"m3_{hh}")
                  for hh in range(n_half)]
            for h in range(n_half):
                nc.tensor.matmul(m3[h], lhsT=w3t_sb,
                                 rhs=h2[:, h * 512:(h + 1) * 512],
                                 start=True, stop=True)
            h3 = hp.tile([D_OUT, rows], f32, tag="h3")
            for h in range(n_half):
                nc.scalar.activation(h3[:, h * 512:(h + 1) * 512], m3[h],
                                     AF.Identity, bias=b3_sb)
            nc.sync.dma_start(out_d, h3)

    nc.compile()
    return nc


def host_prep(x, adj, W_gat, a, gamma, beta, W1, b1, W2, b2, W3, b3,
              num_cores=N_CORES):
    bf16 = ml_dtypes.bfloat16
    n = x.shape[0]
    rows = n // num_cores
    n_chunk = n // P
    n_sc = max(1, n_chunk // SC_CHUNKS)
    sc_chunks = n_chunk // n_sc
    Wh = (x @ W_gat.T).astype(np.float32)
    s = (Wh @ a.T).astype(np.float32).ravel()
    u = np.exp(s)
    # chunks on the ACT-Prelu route compute exp(prelu(s_i+s_j)) directly,
    # so their whu rows must NOT carry the e^{s_j} fold
    uf = u.copy()
    for jc in actp_chunks(n_chunk):
        uf[jc * P:(jc + 1) * P] = 1.0
    whu = np.zeros((n, D_AUG), np.float32)
    whu[:, :D_H] = Wh * uf[:, None]
    whu[:, 64] = uf
    whu_r = np.ascontiguousarray(
        whu.reshape(n_chunk, P, D_AUG).transpose(1, 0, 2)
        .reshape(P, n_chunk * D_AUG)).astype(bf16)
    sJm = np.ascontiguousarray((-4.0 * s).reshape(n_chunk, P).T)
    sJp = np.ascontiguousarray(s.reshape(n_chunk, P).T)
    s_bf = s.astype(bf16).astype(np.float32)
    # fold LayerNorm gamma/beta into the first MLP layer
    W1g = (W1 * gamma[None, :]).astype(np.float32)
    b1g = (b1 + W1 @ beta).astype(np.float32)
    adjT = np.ascontiguousarray(adj.T)  # adjT[j, i] = adj[i, j]
    in_maps = []
    for c in range(num_cores):
        r = slice(c * rows, (c + 1) * rows)
        M = np.where(adjT[:, r] > 0, s_bf[r][None, :],
                     np.float32(MASK_VAL)).astype(bf16)
        M = np.ascontiguousarray(
            M.reshape(n_sc, sc_chunks, P, rows).transpose(0, 2, 1, 3)
            .reshape(n_sc * P, sc_chunks * rows))
        in_maps.append({
            "adjm": M,
            "whu": whu_r,
            "sJm": sJm,
            "sJp": sJp,
            "w1g": np.ascontiguousarray(W1g.T).astype(bf16),
            "b1": np.ascontiguousarray(b1g[:, None]).astype(np.float32),
            "w2t": np.ascontiguousarray(W2.T).astype(bf16),
            "b2": np.ascontiguousarray(b2[:, None]).astype(np.float32),
            "w3t": np.ascontiguousarray(W3.T).astype(bf16),
            "b3": np.ascontiguousarray(b3[:, None]).astype(np.float32),
        })
    return in_maps


def unpack_out(out_t):
    """[D_OUT, rows] transposed output -> [rows, D_OUT]."""
    return np.ascontiguousarray(np.asarray(out_t).T)


_NC_CACHE = {}


def kernel(x, adj, W_gat, a, gamma, beta, W1, b1, W2, b2, W3, b3,
           trace=False):
    from concourse.bass_utils import run_bass_kernel_spmd

    args = [np.asarray(t) for t in
            (x, adj, W_gat, a, gamma, beta, W1, b1, W2, b2, W3, b3)]
    in_maps = host_prep(*args)
    if "nc" not in _NC_CACHE:
        _NC_CACHE["nc"] = build_nc()
    nc = _NC_CACHE["nc"]
    res = run_bass_kernel_spmd(nc, in_maps, list(range(N_CORES)), trace=trace)
    out = np.concatenate([unpack_out(r["out"]) for r in res.results], axis=0)
    if trace:
        kernel.last_results = res
    return out.astype(np.float32)
